# revision 1
# baseline (speedup 1.0000x reference)
"""GAT (2-layer, 4-head + 1-head) + global mean pool + linear head on 8 TRN2 cores.

Strategy (per sharding hint): nodes (and their incident edges, partitioned by
dst) are sharded across 8 cores; small weights replicated. The dense feature
transform h1 = x @ W1 is replicated on every core (cheaper than all-gathering
h1); per-edge work is 1/8 per core.

Phase A (launch 1): dense1 (h1 + attention logits via augmented weights) ->
  per-window (128 dst nodes) layer-1 edge attention: dma_gather of h1[src]
  rows from an int16-safe pair of table halves, indicator-matrix matmuls for
  per-dst softmax denominators and aggregation -> dense2 (h2 + layer-2
  logits). Outputs per-core T2 rows (h2 | al_src2) and per-edge al_dst2.
Phase B (launch 2): layer-2 edge attention (gather h2[src]) -> global mean
  pool partials -> partial logits [64, 2]. Host sums the 8 partials + bl.

Host work is limited to sharding/layout prep (edge sort/partition, index
lists, transposes/padding/dtype casts, per-graph node counts) and unshard
(concat of T2 rows between phases, sum of partial logits).
"""

import contextlib
import hashlib
import os
import numpy as np
import ml_dtypes

import concourse.bass as bass
import concourse.mybir as mybir
import concourse.tile as tile
from concourse import bacc
from concourse import bass_utils
from concourse.masks import make_identity

bf16 = ml_dtypes.bfloat16
F32 = mybir.dt.float32
BF16 = mybir.dt.bfloat16
I16 = mybir.dt.int16
AF = mybir.ActivationFunctionType
ALU = mybir.AluOpType

# ---- problem constants ----
N_NODES = 50000
N_GRAPHS = 64
F_IN = 500
F_IN_PAD = 512
H1 = 256          # heads*hid layer 1
HEADS = 4
HID = 64
NEG_SLOPE = 0.2
NCORES = 8
OWN = N_NODES // NCORES          # 6250
P = 128
NODES_PAD = 50048                # 391*128
NBLK = NODES_PAD // P            # 391
WINDOWS = (OWN + P - 1) // P     # 49
LAST_ROWS = OWN - (WINDOWS - 1) * P   # 106
OWNPAD = WINDOWS * P             # 6272
TAB_HALF = 195 * P               # 24960: block-aligned int16-safe table split
TAB_A = TAB_HALF                 # rows in table A
TAB_B = NODES_PAD - TAB_HALF     # 25088 rows in table B (< 32767)
T1_COLS = 384                    # bf16 row: [as(4) | ad(4) | h1(256) | junk(120)]
T2_COLS = 128                    # f32 row:  [as2(1) | h2(64) | junk(63)]
EPS = 1e-16

TRACE = bool(int(os.environ.get("KERNEL_TRACE", "0")))
MAXWIN = int(os.environ.get("KERNEL_MAXWIN", str(WINDOWS)))
SKIP_DENSE_STORE = bool(int(os.environ.get("KERNEL_SKIP_DENSE", "0")))
SKIP_OWN = bool(int(os.environ.get("KERNEL_SKIP_OWN", "0")))
LAST_TIMES = {}

_CACHE = {}


# ======================================================================
# host preprocessing
# ======================================================================

def _wrap_idx(idx, L):
    pad = np.zeros(L, np.int32)
    pad[: len(idx)] = idx
    return pad.reshape(L // 16, 16).T.astype(np.int16)  # [16, L/16]


def _prep(edge_index, batch):
    src = np.concatenate([edge_index[0], np.arange(N_NODES, dtype=np.int64)])
    dst = np.concatenate([edge_index[1], np.arange(N_NODES, dtype=np.int64)])
    src = src.astype(np.int32)
    dst = dst.astype(np.int32)

    coreinfo = []
    nA = np.zeros((NCORES, WINDOWS), np.int64)
    nB = np.zeros((NCORES, WINDOWS), np.int64)
    for k in range(NCORES):
        m = (dst >= k * OWN) & (dst < (k + 1) * OWN)
        s = src[m]
        d = dst[m] - k * OWN
        w = d >> 7
        order = np.lexsort((s, w))
        s, d, w = s[order], d[order], w[order]
        isA = s < TAB_HALF
        wins = []
        wstart = np.searchsorted(w, np.arange(WINDOWS + 1))
        for wi in range(WINDOWS):
            sl = slice(wstart[wi], wstart[wi + 1])
            sw, dw, aw = s[sl], d[sl], isA[sl]
            wins.append((sw[aw], dw[aw] - wi * P, sw[~aw] - TAB_HALF,
                         dw[~aw] - wi * P))
            nA[k, wi] = int(aw.sum())
            nB[k, wi] = int((~aw).sum())
        coreinfo.append(wins)

    mA = [max(1, int(np.ceil(nA[:, w].max() / P))) for w in range(WINDOWS)]
    mB = [max(1, int(np.ceil(nB[:, w].max() / P))) for w in range(WINDOWS)]
    mW = [a + b for a, b in zip(mA, mB)]
    dims = dict(mA=mA, mB=mB, mW=mW,
                sumA=sum(mA) * P, sumB=sum(mB) * P,
                sumM=sum(mW), sumE=sum(mW) * P, mmax=max(mW))

    per_core = []
    for k in range(NCORES):
        idxA = np.zeros((16, dims["sumA"] // 16), np.int16)
        idxB = np.zeros((16, dims["sumB"] // 16), np.int16)
        dstcol = np.full((dims["sumM"], P), -1.0, bf16)
        maskc = np.zeros((dims["sumM"], P), bf16)
        cA = cB = cM = 0
        for w in range(WINDOWS):
            sA, dA, sB, dB = coreinfo[k][w]
            LA, LB = mA[w] * P, mB[w] * P
            idxA[:, cA // 16:(cA + LA) // 16] = _wrap_idx(sA, LA)
            idxB[:, cB // 16:(cB + LB) // 16] = _wrap_idx(sB, LB)
            dv = np.full(LA + LB, -1.0, np.float32)
            dv[: len(dA)] = dA
            dv[LA: LA + len(dB)] = dB
            mv = np.zeros(LA + LB, np.float32)
            mv[: len(dA)] = 1.0
            mv[LA: LA + len(dB)] = 1.0
            dstcol[cM:cM + mW[w]] = dv.reshape(mW[w], P).astype(bf16)
            maskc[cM:cM + mW[w]] = mv.reshape(mW[w], P).astype(bf16)
            cA += LA
            cB += LB
            cM += mW[w]

        bv = np.full((OWNPAD,), -1.0, np.float32)
        bv[:OWN] = batch[k * OWN:(k + 1) * OWN].astype(np.float32)
        # own-node table row ids (for al_dst of own windows), A/B split + select
        own = np.arange(OWNPAD, dtype=np.int32) + k * OWN
        own = np.minimum(own, NODES_PAD - 1)
        selA = own < TAB_HALF
        ownA = np.where(selA, own, 0)
        ownB = np.where(selA, 0, own - TAB_HALF)
        sel = selA.astype(np.float32).reshape(WINDOWS, P).astype(bf16)
        per_core.append(dict(
            idxA=idxA, idxB=idxB, dstcol=dstcol, maskc=maskc,
            batchv=bv.astype(bf16),
            ownA=_wrap_idx(ownA, OWNPAD), ownB=_wrap_idx(ownB, OWNPAD),
            ownsel=sel))
    return dims, per_core


def _prep_weights(x, W1, a_src1, a_dst1, W2, a_src2, a_dst2):
    xT = np.zeros((F_IN_PAD, NODES_PAD), bf16)
    xT[:F_IN, :N_NODES] = x.T.astype(bf16)

    Asrc = np.zeros((H1, HEADS), np.float32)
    Adst = np.zeros((H1, HEADS), np.float32)
    for h in range(HEADS):
        Asrc[h * HID:(h + 1) * HID, h] = a_src1[h]
        Adst[h * HID:(h + 1) * HID, h] = a_dst1[h]
    Waug = np.zeros((F_IN_PAD, 8 + H1), np.float32)
    Waug[:F_IN, 0:4] = W1 @ Asrc
    Waug[:F_IN, 4:8] = W1 @ Adst
    Waug[:F_IN, 8:] = W1
    Waug = Waug.astype(bf16)

    W2aug = np.zeros((H1, HID + 2), np.float32)
    W2aug[:, :HID] = W2
    W2aug[:, HID] = W2 @ a_src2[0]
    W2aug[:, HID + 1] = W2 @ a_dst2[0]
    W2aug = W2aug.astype(bf16)
    return xT, Waug, W2aug


# ======================================================================
# phase A builder
# ======================================================================

def build_phase_a(dims):
    mA, mB, mW = dims["mA"], dims["mB"], dims["mW"]
    mmax = dims["mmax"]
    nc = bacc.Bacc("TRN2", target_bir_lowering=False, debug=False)

    xT_d = nc.dram_tensor("xT", [F_IN_PAD, NODES_PAD], BF16, kind="ExternalInput")
    Waug_d = nc.dram_tensor("Waug", [F_IN_PAD, 264], BF16, kind="ExternalInput")
    W2aug_d = nc.dram_tensor("W2aug", [H1, 66], BF16, kind="ExternalInput")
    idxA_d = nc.dram_tensor("idxA", [16, dims["sumA"] // 16], I16, kind="ExternalInput")
    idxB_d = nc.dram_tensor("idxB", [16, dims["sumB"] // 16], I16, kind="ExternalInput")
    dstcol_d = nc.dram_tensor("dstcol", [dims["sumM"], P], BF16, kind="ExternalInput")
    maskc_d = nc.dram_tensor("maskc", [dims["sumM"], P], BF16, kind="ExternalInput")
    ownA_d = nc.dram_tensor("ownA", [16, OWNPAD // 16], I16, kind="ExternalInput")
    ownB_d = nc.dram_tensor("ownB", [16, OWNPAD // 16], I16, kind="ExternalInput")
    ownsel_d = nc.dram_tensor("ownsel", [WINDOWS, P], BF16, kind="ExternalInput")
    iotaF_d = nc.dram_tensor("iotaF", [1, P], BF16, kind="ExternalInput")
    iotaC_d = nc.dram_tensor("iotaC", [P, 1], BF16, kind="ExternalInput")
    b1_d = nc.dram_tensor("b1r", [1, H1], F32, kind="ExternalInput")

    T2own_d = nc.dram_tensor("T2own", [OWNPAD, 65], F32, kind="ExternalOutput")
    ad2_d = nc.dram_tensor("ad2", [dims["sumE"]], F32, kind="ExternalOutput")

    with tile.TileContext(nc) as tc:
        ctx = contextlib.ExitStack()
        with ctx:
            dram = ctx.enter_context(tc.tile_pool(name="dram", bufs=1, space="DRAM"))
            T1a = dram.tile([TAB_A, T1_COLS], BF16)
            T1b = dram.tile([TAB_B, T1_COLS], BF16)

            const = ctx.enter_context(tc.tile_pool(name="const", bufs=1))
            waug_t = const.tile([P, 4, 264], BF16)
            nc.sync.dma_start(waug_t[:], Waug_d[:].rearrange("(ko p) c -> p ko c", p=P))
            w2aug_t = const.tile([P, 2, 66], BF16)
            nc.sync.dma_start(w2aug_t[:], W2aug_d[:].rearrange("(ko p) c -> p ko c", p=P))
            iotaF_t = const.tile([P, P], BF16)
            nc.sync.dma_start(iotaF_t[:], iotaF_d[:].to_broadcast([P, P]))
            iotaC_t = const.tile([P, 1], BF16)
            nc.sync.dma_start(iotaC_t[:], iotaC_d[:])
            b1_t = const.tile([P, H1], F32)
            nc.sync.dma_start(b1_t[:], b1_d[:].to_broadcast([P, H1]))
            ident_t = const.tile([P, P], F32)
            make_identity(nc, ident_t[:])
            ones_t = const.tile([1, P], BF16)
            nc.vector.memset(ones_t[:], 1.0)
            # own-node [as|ad] cache, filled after dense phase
            ocp = const.tile([P, WINDOWS, 8], BF16)

            # ---------------- dense phase ----------------
            CH = 8  # node blocks per xT chunk
            with tc.tile_pool(name="dense", bufs=3) as dpool, \
                 tc.tile_pool(name="dpsum", bufs=2, space="PSUM") as dps:
                for c0 in range(0, NBLK, CH):
                    nchunk = min(CH, NBLK - c0) * P
                    xt_t = dpool.tile([P, 4, CH * P], BF16, tag="xt")
                    nc.sync.dma_start(
                        xt_t[:, :, :nchunk],
                        xT_d[:].rearrange("(ko p) n -> p ko n", p=P)[
                            :, :, c0 * P: c0 * P + nchunk],
                    )
                    for b in range(nchunk // P):
                        ps = dps.tile([P, 264], F32, tag="dps")
                        for ko in range(4):
                            nc.tensor.matmul(
                                ps[:],
                                lhsT=xt_t[:, ko, b * P:(b + 1) * P],
                                rhs=waug_t[:, ko, :],
                                start=(ko == 0),
                                stop=(ko == 3),
                            )
                        t1_t = dpool.tile([P, 264], BF16, tag="t1")
                        nc.scalar.copy(t1_t[:], ps[:])
                        nb = c0 + b
                        if nb < 195:
                            nc.sync.dma_start(
                                T1a[nb * P:(nb + 1) * P, 0:264], t1_t[:])
                        else:
                            r0 = nb * P - TAB_A
                            nc.sync.dma_start(
                                T1b[r0:r0 + P, 0:264], t1_t[:])

            # own [as|ad] rows via A/B gather + select (program is
            # core-independent; indices/select are per-core data)
            if SKIP_OWN:
                nc.vector.memset(ocp[:], 0.0)
            else:
              with tc.tile_pool(name="own", bufs=1) as opool:
                  oiA = opool.tile([P, OWNPAD // 16], I16, tag="oiA")
                  nc.sync.dma_start(
                      oiA[:], ownA_d[None, :, :].to_broadcast([8, 16, OWNPAD // 16]))
                  oiB = opool.tile([P, OWNPAD // 16], I16, tag="oiB")
                  nc.sync.dma_start(
                      oiB[:], ownB_d[None, :, :].to_broadcast([8, 16, OWNPAD // 16]))
                  ogA = opool.tile([P, WINDOWS, T1_COLS], BF16, tag="ogA")
                  nc.gpsimd.dma_gather(
                      out_ap=ogA[:], in_ap=T1a[:], idxs_ap=oiA[:],
                      num_idxs=OWNPAD, num_idxs_reg=OWNPAD, elem_size=T1_COLS,
                    single_packet=False)
                  ogB = opool.tile([P, WINDOWS, T1_COLS], BF16, tag="ogB")
                  nc.gpsimd.dma_gather(
                      out_ap=ogB[:], in_ap=T1b[:], idxs_ap=oiB[:],
                      num_idxs=OWNPAD, num_idxs_reg=OWNPAD, elem_size=T1_COLS,
                    single_packet=False)
                  osel = opool.tile([P, WINDOWS], BF16, tag="osel")
                  nc.sync.dma_start(osel[:], ownsel_d[:].rearrange("j p -> p j"))
                  oinv = opool.tile([P, WINDOWS], BF16, tag="oinv")
                  nc.vector.tensor_scalar(
                      oinv[:], osel[:], -1.0, 1.0, ALU.mult, ALU.add)
                  tmpA = opool.tile([P, WINDOWS, 8], BF16, tag="tmpA")
                  nc.vector.tensor_tensor(
                      tmpA[:], ogA[:, :, 0:8],
                      osel[:, :, None].to_broadcast([P, WINDOWS, 8]), ALU.mult)
                  tmpB = opool.tile([P, WINDOWS, 8], BF16, tag="tmpB")
                  nc.vector.tensor_tensor(
                      tmpB[:], ogB[:, :, 0:8],
                      oinv[:, :, None].to_broadcast([P, WINDOWS, 8]), ALU.mult)
                  nc.vector.tensor_tensor(ocp[:], tmpA[:], tmpB[:], ALU.add)

            # ---------------- window loop (layer 1 + dense 2) ----------------
            wpool = ctx.enter_context(tc.tile_pool(name="win", bufs=2))
            spool = ctx.enter_context(tc.tile_pool(name="small", bufs=2))
            ps_dr = ctx.enter_context(tc.tile_pool(name="psdr", bufs=2, space="PSUM"))
            ps_ad1 = ctx.enter_context(tc.tile_pool(name="psad1", bufs=1, space="PSUM"))
            ps_agg = ctx.enter_context(tc.tile_pool(name="psagg", bufs=2, space="PSUM"))
            ps_z1t = ctx.enter_context(tc.tile_pool(name="psz1t", bufs=1, space="PSUM"))
            ps_h2 = ctx.enter_context(tc.tile_pool(name="psh2", bufs=1, space="PSUM"))
            ps_ad2 = ctx.enter_context(tc.tile_pool(name="psad2", bufs=1, space="PSUM"))

            cA = cB = cM = cE = 0
            for w in range(WINDOWS):
                ma, mb, m = mA[w], mB[w], mW[w]
                Ew = m * P
                rows = LAST_ROWS if w == WINDOWS - 1 else P
                if w >= MAXWIN:
                    cA += ma * P; cB += mb * P; cM += m; cE += Ew
                    continue

                # --- loads ---
                ia_t = wpool.tile([P, 8 * mmax], I16, tag="ia")
                nc.sync.dma_start(
                    ia_t[:, : 8 * ma],
                    idxA_d[None, :, cA // 16:(cA + ma * P) // 16]
                    .to_broadcast([8, 16, 8 * ma]))
                ib_t = wpool.tile([P, 8 * mmax], I16, tag="ib")
                nc.sync.dma_start(
                    ib_t[:, : 8 * mb],
                    idxB_d[None, :, cB // 16:(cB + mb * P) // 16]
                    .to_broadcast([8, 16, 8 * mb]))
                v_t = wpool.tile([P, mmax, T1_COLS], BF16, tag="v")
                nc.gpsimd.dma_gather(
                    out_ap=v_t[:, 0:ma, :], in_ap=T1a[:],
                    idxs_ap=ia_t[:, : 8 * ma],
                    num_idxs=ma * P, num_idxs_reg=ma * P, elem_size=T1_COLS,
                    single_packet=False)
                nc.gpsimd.dma_gather(
                    out_ap=v_t[:, ma:m, :], in_ap=T1b[:],
                    idxs_ap=ib_t[:, : 8 * mb],
                    num_idxs=mb * P, num_idxs_reg=mb * P, elem_size=T1_COLS,
                    single_packet=False)
                dcol_t = wpool.tile([P, mmax], BF16, tag="dcol")
                nc.sync.dma_start(
                    dcol_t[:, :m], dstcol_d[cM:cM + m, :].rearrange("j p -> p j"))
                msk_t = wpool.tile([P, mmax], BF16, tag="msk")
                nc.sync.dma_start(
                    msk_t[:, :m], maskc_d[cM:cM + m, :].rearrange("j p -> p j"))
                drow_t = wpool.tile([1, mmax * P], BF16, tag="drow")
                nc.sync.dma_start(
                    drow_t[:, :Ew],
                    dstcol_d[cM:cM + m, :].rearrange("j p -> (j p)")[None, :])

                # --- S (edge-major indicator) ---
                s_t = wpool.tile([P, mmax, P], BF16, tag="s")
                nc.vector.tensor_tensor(
                    s_t[:, :m, :],
                    dcol_t[:, :m, None].to_broadcast([P, m, P]),
                    iotaF_t[:, None, :].to_broadcast([P, m, P]),
                    ALU.is_equal)
                # --- S_T (dst-major indicator) via PE row-broadcast ---
                drb_t = wpool.tile([P, mmax * P], BF16, tag="drb")
                for c0 in range(0, Ew, 512):
                    cw = min(512, Ew - c0)
                    psd = ps_dr.tile([P, 512], F32, tag="psdr")
                    nc.tensor.matmul(
                        psd[:, :cw], lhsT=ones_t[:], rhs=drow_t[:, c0:c0 + cw],
                        start=True, stop=True)
                    nc.scalar.copy(drb_t[:, c0:c0 + cw], psd[:, :cw])
                str_t = wpool.tile([P, mmax * P], BF16, tag="str")
                nc.vector.tensor_tensor(
                    str_t[:, :Ew],
                    iotaC_t[:].to_broadcast([P, Ew]),
                    drb_t[:, :Ew],
                    ALU.is_equal)

                # --- ad1 per edge ---
                pad1 = ps_ad1.tile([P, 4 * mmax], F32, tag="psad1")
                for j in range(m):
                    nc.tensor.matmul(
                        pad1[:, j * 4:(j + 1) * 4],
                        lhsT=str_t[:, j * P:(j + 1) * P],
                        rhs=ocp[:, w, 4:8],
                        start=True, stop=True)
                # --- ex = exp(lrelu(as + ad)) * mask ---
                zf = spool.tile([P, mmax, 4], F32, tag="zf")
                nc.vector.tensor_tensor(
                    zf[:, :m, :], v_t[:, :m, 0:4],
                    pad1[:].rearrange("p (j c) -> p j c", c=4)[:, :m, :],
                    ALU.add)
                zt = spool.tile([P, mmax, 4], F32, tag="zt")
                nc.vector.tensor_scalar_mul(zt[:, :m, :], zf[:, :m, :], NEG_SLOPE)
                nc.vector.tensor_tensor(zt[:, :m, :], zt[:, :m, :], zf[:, :m, :],
                                        ALU.max)
                ex_t = spool.tile([P, mmax, 4], BF16, tag="ex")
                nc.scalar.activation(ex_t[:, :m, :], zt[:, :m, :], AF.Exp)
                nc.vector.tensor_tensor(
                    ex_t[:, :m, :], ex_t[:, :m, :],
                    msk_t[:, :m, None].to_broadcast([P, m, 4]), ALU.mult)
                # --- Vw = [h*ex | ex] ---
                vw_t = wpool.tile([P, mmax, 260], BF16, tag="vw")
                nc.vector.tensor_tensor(
                    vw_t[:, :m, 0:256].rearrange("p m (h c) -> p m h c", h=HEADS),
                    v_t[:, :m, 8:264].rearrange("p m (h c) -> p m h c", h=HEADS),
                    ex_t[:, :m, :, None].to_broadcast([P, m, HEADS, HID]),
                    ALU.mult)
                nc.vector.tensor_copy(vw_t[:, :m, 256:260], ex_t[:, :m, :])

                # --- aggregate ---
                pagg = ps_agg.tile([P, 260], F32, tag="psagg")
                for j in range(m):
                    nc.tensor.matmul(
                        pagg[:], lhsT=s_t[:, j, :], rhs=vw_t[:, j, :],
                        start=(j == 0), stop=(j == m - 1))
                # --- out1 = agg / s + b1 ; z1 = relu ---
                sden = spool.tile([P, 4], F32, tag="sden")
                nc.vector.tensor_scalar_add(sden[:], pagg[:, 256:260], EPS)
                nc.vector.reciprocal(sden[:], sden[:])
                z1 = spool.tile([P, H1], F32, tag="z1")
                nc.vector.tensor_tensor(
                    z1[:].rearrange("p (h c) -> p h c", h=HEADS),
                    pagg[:, 0:256].rearrange("p (h c) -> p h c", h=HEADS),
                    sden[:, :, None].to_broadcast([P, HEADS, HID]),
                    ALU.mult)
                nc.vector.tensor_add(z1[:], z1[:], b1_t[:])
                nc.scalar.activation(z1[:], z1[:], AF.Relu)

                # --- dense 2: h2aug = z1 @ W2aug ---
                z1t = spool.tile([P, 2, P], BF16, tag="z1t")
                for hh in range(2):
                    pzt = ps_z1t.tile([P, P], F32, tag="psz1t")
                    nc.tensor.transpose(
                        pzt[:], z1[:, hh * P:(hh + 1) * P], ident_t[:])
                    nc.scalar.copy(z1t[:, hh, :], pzt[:])
                ph2 = ps_h2.tile([P, 66], F32, tag="psh2")
                for hh in range(2):
                    nc.tensor.matmul(
                        ph2[:], lhsT=z1t[:, hh, :], rhs=w2aug_t[:, hh, :],
                        start=(hh == 0), stop=(hh == 1))
                t2_t = spool.tile([P, 65], F32, tag="t2")
                nc.scalar.copy(t2_t[:, 0:1], ph2[:, 64:65])
                nc.scalar.copy(t2_t[:, 1:65], ph2[:, 0:64])
                nc.sync.dma_start(
                    T2own_d[w * P: w * P + rows, :], t2_t[:rows, :])

                # --- ad2 per edge (for phase B) ---
                ald2 = spool.tile([P, 1], BF16, tag="ald2")
                nc.scalar.copy(ald2[:], ph2[:, 65:66])
                pad2 = ps_ad2.tile([P, mmax], F32, tag="psad2")
                for j in range(m):
                    nc.tensor.matmul(
                        pad2[:, j:j + 1],
                        lhsT=str_t[:, j * P:(j + 1) * P],
                        rhs=ald2[:], start=True, stop=True)
                ad2s = spool.tile([P, mmax], F32, tag="ad2s")
                nc.vector.tensor_copy(ad2s[:, :m], pad2[:, :m])
                nc.sync.dma_start(
                    ad2_d[cE:cE + Ew].rearrange("(j p) -> p j", p=P),
                    ad2s[:, :m])

                cA += ma * P
                cB += mb * P
                cM += m
                cE += Ew

    nc.compile()
    return nc


# ======================================================================
# phase B builder
# ======================================================================

def build_phase_b(dims):
    mA, mB, mW = dims["mA"], dims["mB"], dims["mW"]
    mmax = dims["mmax"]
    nc = bacc.Bacc("TRN2", target_bir_lowering=False, debug=False)

    T2A_d = nc.dram_tensor("T2A", [TAB_A, T2_COLS], F32, kind="ExternalInput")
    T2B_d = nc.dram_tensor("T2B", [TAB_B, T2_COLS], F32, kind="ExternalInput")
    idxA_d = nc.dram_tensor("idxA", [16, dims["sumA"] // 16], I16, kind="ExternalInput")
    idxB_d = nc.dram_tensor("idxB", [16, dims["sumB"] // 16], I16, kind="ExternalInput")
    dstcol_d = nc.dram_tensor("dstcol", [dims["sumM"], P], BF16, kind="ExternalInput")
    maskc_d = nc.dram_tensor("maskc", [dims["sumM"], P], BF16, kind="ExternalInput")
    ad2_d = nc.dram_tensor("ad2", [dims["sumE"]], F32, kind="ExternalInput")
    iotaF_d = nc.dram_tensor("iotaF", [1, P], BF16, kind="ExternalInput")
    giota_d = nc.dram_tensor("giota", [1, N_GRAPHS], BF16, kind="ExternalInput")
    batchv_d = nc.dram_tensor("batchv", [OWNPAD], BF16, kind="ExternalInput")
    b2_d = nc.dram_tensor("b2r", [1, HID], F32, kind="ExternalInput")
    cnt_d = nc.dram_tensor("cnt", [N_GRAPHS, 1], F32, kind="ExternalInput")
    Wl_d = nc.dram_tensor("Wl", [HID, 2], F32, kind="ExternalInput")

    out_d = nc.dram_tensor("partial", [N_GRAPHS, 2], F32, kind="ExternalOutput")

    with tile.TileContext(nc) as tc:
        ctx = contextlib.ExitStack()
        with ctx:
            const = ctx.enter_context(tc.tile_pool(name="const", bufs=1))
            iotaF_t = const.tile([P, P], BF16)
            nc.sync.dma_start(iotaF_t[:], iotaF_d[:].to_broadcast([P, P]))
            giota_t = const.tile([P, N_GRAPHS], BF16)
            nc.sync.dma_start(giota_t[:], giota_d[:].to_broadcast([P, N_GRAPHS]))
            b2_t = const.tile([P, HID], F32)
            nc.sync.dma_start(b2_t[:], b2_d[:].to_broadcast([P, HID]))
            cnt_t = const.tile([N_GRAPHS, 1], F32)
            nc.sync.dma_start(cnt_t[:], cnt_d[:])
            wl_t = const.tile([P, 2], F32)
            nc.vector.memset(wl_t[:], 0.0)
            nc.sync.dma_start(wl_t[:HID, :], Wl_d[:])
            ident_t = const.tile([P, P], F32)
            make_identity(nc, ident_t[:])
            pts = const.tile([P, N_GRAPHS], F32)
            nc.vector.memset(pts[:], 0.0)

            wpool = ctx.enter_context(tc.tile_pool(name="win", bufs=2))
            spool = ctx.enter_context(tc.tile_pool(name="small", bufs=2))
            ps_agg = ctx.enter_context(tc.tile_pool(name="psagg", bufs=2, space="PSUM"))
            ps_pool = ctx.enter_context(tc.tile_pool(name="pspool", bufs=1, space="PSUM"))
            ps_fin = ctx.enter_context(tc.tile_pool(name="psfin", bufs=1, space="PSUM"))

            ppool = ps_pool.tile([N_GRAPHS, HID], F32)

            cA = cB = cM = cE = 0
            for w in range(WINDOWS):
                ma, mb, m = mA[w], mB[w], mW[w]
                Ew = m * P

                ia_t = wpool.tile([P, 8 * mmax], I16, tag="ia")
                nc.sync.dma_start(
                    ia_t[:, : 8 * ma],
                    idxA_d[None, :, cA // 16:(cA + ma * P) // 16]
                    .to_broadcast([8, 16, 8 * ma]))
                ib_t = wpool.tile([P, 8 * mmax], I16, tag="ib")
                nc.sync.dma_start(
                    ib_t[:, : 8 * mb],
                    idxB_d[None, :, cB // 16:(cB + mb * P) // 16]
                    .to_broadcast([8, 16, 8 * mb]))
                v_t = wpool.tile([P, mmax, T2_COLS], F32, tag="v")
                nc.gpsimd.dma_gather(
                    out_ap=v_t[:, 0:ma, :], in_ap=T2A_d[:],
                    idxs_ap=ia_t[:, : 8 * ma],
                    num_idxs=ma * P, num_idxs_reg=ma * P, elem_size=T2_COLS,
                    single_packet=False)
                nc.gpsimd.dma_gather(
                    out_ap=v_t[:, ma:m, :], in_ap=T2B_d[:],
                    idxs_ap=ib_t[:, : 8 * mb],
                    num_idxs=mb * P, num_idxs_reg=mb * P, elem_size=T2_COLS,
                    single_packet=False)
                dcol_t = wpool.tile([P, mmax], BF16, tag="dcol")
                nc.sync.dma_start(
                    dcol_t[:, :m], dstcol_d[cM:cM + m, :].rearrange("j p -> p j"))
                msk_t = wpool.tile([P, mmax], BF16, tag="msk")
                nc.sync.dma_start(
                    msk_t[:, :m], maskc_d[cM:cM + m, :].rearrange("j p -> p j"))
                ad2_t = wpool.tile([P, mmax], F32, tag="ad2")
                nc.sync.dma_start(
                    ad2_t[:, :m],
                    ad2_d[cE:cE + Ew].rearrange("(j p) -> p j", p=P))
                bv_t = spool.tile([P, 1], BF16, tag="bv")
                nc.sync.dma_start(bv_t[:], batchv_d[w * P:(w + 1) * P, None])

                s_t = wpool.tile([P, mmax, P], BF16, tag="s")
                nc.vector.tensor_tensor(
                    s_t[:, :m, :],
                    dcol_t[:, :m, None].to_broadcast([P, m, P]),
                    iotaF_t[:, None, :].to_broadcast([P, m, P]),
                    ALU.is_equal)
                zf = spool.tile([P, mmax], F32, tag="zf")
                nc.vector.tensor_tensor(
                    zf[:, :m], v_t[:, :m, 0], ad2_t[:, :m], ALU.add)
                zt = spool.tile([P, mmax], F32, tag="zt")
                nc.vector.tensor_scalar_mul(zt[:, :m], zf[:, :m], NEG_SLOPE)
                nc.vector.tensor_tensor(zt[:, :m], zt[:, :m], zf[:, :m], ALU.max)
                ex_t = spool.tile([P, mmax], BF16, tag="ex")
                nc.scalar.activation(ex_t[:, :m], zt[:, :m], AF.Exp)
                nc.vector.tensor_tensor(
                    ex_t[:, :m], ex_t[:, :m], msk_t[:, :m], ALU.mult)

                vw_t = wpool.tile([P, mmax, 65], BF16, tag="vw")
                nc.vector.tensor_tensor(
                    vw_t[:, :m, 0:64],
                    v_t[:, :m, 1:65],
                    ex_t[:, :m, None].to_broadcast([P, m, HID]),
                    ALU.mult)
                nc.vector.tensor_copy(vw_t[:, :m, 64:65], ex_t[:, :m, None])

                pagg = ps_agg.tile([P, 65], F32, tag="psagg")
                for j in range(m):
                    nc.tensor.matmul(
                        pagg[:], lhsT=s_t[:, j, :], rhs=vw_t[:, j, :],
                        start=(j == 0), stop=(j == m - 1))
                sden = spool.tile([P, 1], F32, tag="sden")
                nc.vector.tensor_scalar_add(sden[:], pagg[:, 64:65], EPS)
                nc.vector.reciprocal(sden[:], sden[:])
                z2 = spool.tile([P, HID], F32, tag="z2")
                nc.vector.tensor_tensor(
                    z2[:], pagg[:, 0:64], sden[:].to_broadcast([P, HID]),
                    ALU.mult)
                nc.vector.tensor_add(z2[:], z2[:], b2_t[:])
                z2b = spool.tile([P, HID], BF16, tag="z2b")
                nc.scalar.activation(z2b[:], z2[:], AF.Relu)

                pw_t = spool.tile([P, N_GRAPHS], BF16, tag="pw")
                nc.vector.tensor_tensor(
                    pw_t[:], bv_t[:].to_broadcast([P, N_GRAPHS]), giota_t[:],
                    ALU.is_equal)
                nc.tensor.matmul(
                    ppool[:], lhsT=pw_t[:], rhs=z2b[:],
                    start=(w == 0), stop=(w == WINDOWS - 1))
                cA += ma * P
                cB += mb * P
                cM += m
                cE += Ew

            # pooled partial logits
            crec = spool.tile([N_GRAPHS, 1], F32, tag="crec")
            nc.vector.reciprocal(crec[:], cnt_t[:])
            pooled = spool.tile([N_GRAPHS, HID], F32, tag="pooled")
            nc.vector.tensor_tensor(
                pooled[:], ppool[:], crec[:].to_broadcast([N_GRAPHS, HID]),
                ALU.mult)
            ptp = ps_fin.tile([HID, N_GRAPHS], F32)
            nc.tensor.transpose(ptp[:], pooled[:], ident_t[:N_GRAPHS, :N_GRAPHS])
            nc.vector.tensor_copy(pts[:HID, :], ptp[:])
            plog = ps_fin.tile([N_GRAPHS, 2], F32)
            nc.tensor.matmul(plog[:], lhsT=pts[:], rhs=wl_t[:],
                             start=True, stop=True)
            outs = spool.tile([N_GRAPHS, 2], F32, tag="outs")
            nc.vector.tensor_copy(outs[:], plog[:])
            nc.sync.dma_start(out_d[:], outs[:])

    nc.compile()
    return nc


# ======================================================================
# driver
# ======================================================================

def _run(nc, in_maps, label):
    res = bass_utils.run_bass_kernel_spmd(
        nc, in_maps, core_ids=list(range(NCORES)), trace=TRACE)
    if TRACE:
        LAST_TIMES[label] = res.exec_time_ns
    return res.results


def kernel(x, edge_index, batch, W1, a_src1, a_dst1, b1,
           W2, a_src2, a_dst2, b2, Wl, bl):
    if TRACE:
        try:
            import axon_shim  # noqa: F401
        except ImportError:
            pass

    x = np.asarray(x, np.float32)
    edge_index = np.asarray(edge_index)
    batch = np.asarray(batch)

    key = hashlib.sha1(edge_index.tobytes() + batch.tobytes()).hexdigest()
    if key in _CACHE:
        dims, per_core, nc_a, nc_b = _CACHE[key]
    else:
        dims, per_core = _prep(edge_index, batch)
        nc_a = build_phase_a(dims)
        nc_b = build_phase_b(dims)
        _CACHE[key] = (dims, per_core, nc_a, nc_b)

    xT, Waug, W2aug = _prep_weights(
        x, np.asarray(W1, np.float32), np.asarray(a_src1, np.float32),
        np.asarray(a_dst1, np.float32), np.asarray(W2, np.float32),
        np.asarray(a_src2, np.float32), np.asarray(a_dst2, np.float32))

    iotaF = np.arange(P, dtype=np.float32).astype(bf16)[None, :]
    iotaC = np.arange(P, dtype=np.float32).astype(bf16)[:, None]
    giota = np.arange(N_GRAPHS, dtype=np.float32).astype(bf16)[None, :]
    b1r = np.asarray(b1, np.float32)[None, :]
    b2r = np.asarray(b2, np.float32)[None, :]
    cnt = np.maximum(
        np.bincount(np.asarray(batch).astype(np.int64), minlength=N_GRAPHS), 1
    ).astype(np.float32)[:, None]
    Wl32 = np.asarray(Wl, np.float32)
    bl32 = np.asarray(bl, np.float32)

    in_maps_a = []
    for k in range(NCORES):
        pc = per_core[k]
        in_maps_a.append(dict(
            xT=xT, Waug=Waug, W2aug=W2aug,
            idxA=pc["idxA"], idxB=pc["idxB"],
            dstcol=pc["dstcol"], maskc=pc["maskc"],
            ownA=pc["ownA"], ownB=pc["ownB"], ownsel=pc["ownsel"],
            iotaF=iotaF, iotaC=iotaC, b1r=b1r,
        ))
    res_a = _run(nc_a, in_maps_a, "phase_a")

    T2 = np.zeros((NODES_PAD, T2_COLS), np.float32)
    for k in range(NCORES):
        T2[k * OWN:(k + 1) * OWN, 0:65] = res_a[k]["T2own"][:OWN, :]
    T2A, T2B = T2[:TAB_A], T2[TAB_A:]

    in_maps_b = []
    for k in range(NCORES):
        pc = per_core[k]
        in_maps_b.append(dict(
            T2A=T2A, T2B=T2B,
            idxA=pc["idxA"], idxB=pc["idxB"],
            dstcol=pc["dstcol"], maskc=pc["maskc"],
            ad2=res_a[k]["ad2"],
            iotaF=iotaF, giota=giota,
            batchv=pc["batchv"], b2r=b2r, cnt=cnt, Wl=Wl32,
        ))
    res_b = _run(nc_b, in_maps_b, "phase_b")

    out = np.zeros((N_GRAPHS, 2), np.float32)
    for k in range(NCORES):
        out += res_b[k]["partial"]
    out += bl32[None, :]
    return out



# revision 2
# speedup vs baseline: 1.9076x; 1.9076x over previous
"""GAT (2-layer, 4-head + 1-head) + global mean pool + linear head on 8 TRN2 cores.

Strategy (per sharding hint): nodes (and their incident edges, partitioned by
dst) are sharded across 8 cores; small weights replicated. The dense feature
transform h1 = x @ W1 is replicated on every core (cheaper than all-gathering
h1); per-edge work is 1/8 per core.

Phase A (launch 1): dense1 (h1 + attention logits via augmented weights) ->
  per-window (128 dst nodes) layer-1 edge attention: dma_gather of h1[src]
  rows from an int16-safe pair of table halves, indicator-matrix matmuls for
  per-dst softmax denominators and aggregation -> dense2 (h2 + layer-2
  logits). Outputs per-core T2 rows (h2 | al_src2) and per-edge al_dst2.
Phase B (launch 2): layer-2 edge attention (gather h2[src]) -> global mean
  pool partials -> partial logits [64, 2]. Host sums the 8 partials + bl.

Perf notes vs the original version:
- num_swdge_queues=4 + queue_num round-robin: dma_gather descriptor emission
  is pinned to ONE Q7 core pair per queue (ucode cpu_id/2 == queue_num), so
  a single queue serializes all gathers on 2 of 8 Q7 cores.
- Dense-phase T1 stores batched per 8-block chunk (was 391 per-block DMAs,
  each ~1.9us of sync-sequencer fixed cost).
- Per-window idx/dstcol loads hoisted into one up-front DMA each; mask input
  dropped entirely (the dst-indicator matmul already zeroes padding edges);
  ad2 and T2 rows accumulated in SBUF and stored once at the end.
- dstcol consumed from a host-side transposed copy (the old per-window
  "j p -> p j" DMA rearrange generated 2-byte descriptors).
- T2 table in bf16 (256B gather descriptors instead of 512B).

Host work is limited to sharding/layout prep (edge sort/partition, index
lists, transposes/padding/dtype casts, per-graph node counts) and unshard
(concat of T2 rows between phases, sum of partial logits).
"""

import contextlib
import hashlib
import os
import numpy as np
import ml_dtypes

import concourse.bass as bass
import concourse.mybir as mybir
import concourse.tile as tile
from concourse import bacc
from concourse import bass_utils
from concourse.masks import make_identity

bf16 = ml_dtypes.bfloat16
F32 = mybir.dt.float32
BF16 = mybir.dt.bfloat16
I16 = mybir.dt.int16
AF = mybir.ActivationFunctionType
ALU = mybir.AluOpType

# ---- problem constants ----
N_NODES = 50000
N_GRAPHS = 64
F_IN = 500
F_IN_PAD = 512
H1 = 256          # heads*hid layer 1
HEADS = 4
HID = 64
NEG_SLOPE = 0.2
NCORES = 8
OWN = N_NODES // NCORES          # 6250
P = 128
NODES_PAD = 50048                # 391*128
NBLK = NODES_PAD // P            # 391
WINDOWS = (OWN + P - 1) // P     # 49
LAST_ROWS = OWN - (WINDOWS - 1) * P   # 106
OWNPAD = WINDOWS * P             # 6272
TAB_HALF = 195 * P               # 24960: block-aligned int16-safe table split
TAB_A = TAB_HALF                 # rows in table A
TAB_B = NODES_PAD - TAB_HALF     # 25088 rows in table B (< 32767)
TAB_ABLK = TAB_A // P            # 195
T1_COLS = 384                    # bf16 row: [as(4) | ad(4) | h1(256) | junk(120)]
T2_COLS = 128                    # bf16 row: [as2(1) | h2(64) | junk(63)]
EPS = 1e-16

TRACE = bool(int(os.environ.get("KERNEL_TRACE", "0")))
MAXWIN = int(os.environ.get("KERNEL_MAXWIN", str(WINDOWS)))
SKIP_OWN = bool(int(os.environ.get("KERNEL_SKIP_OWN", "0")))
LAST_TIMES = {}

_CACHE = {}


# ======================================================================
# host preprocessing
# ======================================================================

def _wrap_idx(idx, L):
    pad = np.zeros(L, np.int32)
    pad[: len(idx)] = idx
    return pad.reshape(L // 16, 16).T.astype(np.int16)  # [16, L/16]


def _prep(edge_index, batch):
    src = np.concatenate([edge_index[0], np.arange(N_NODES, dtype=np.int64)])
    dst = np.concatenate([edge_index[1], np.arange(N_NODES, dtype=np.int64)])
    src = src.astype(np.int32)
    dst = dst.astype(np.int32)

    coreinfo = []
    nA = np.zeros((NCORES, WINDOWS), np.int64)
    nB = np.zeros((NCORES, WINDOWS), np.int64)
    for k in range(NCORES):
        m = (dst >= k * OWN) & (dst < (k + 1) * OWN)
        s = src[m]
        d = dst[m] - k * OWN
        w = d >> 7
        order = np.lexsort((s, w))
        s, d, w = s[order], d[order], w[order]
        isA = s < TAB_HALF
        wins = []
        wstart = np.searchsorted(w, np.arange(WINDOWS + 1))
        for wi in range(WINDOWS):
            sl = slice(wstart[wi], wstart[wi + 1])
            sw, dw, aw = s[sl], d[sl], isA[sl]
            wins.append((sw[aw], dw[aw] - wi * P, sw[~aw] - TAB_HALF,
                         dw[~aw] - wi * P))
            nA[k, wi] = int(aw.sum())
            nB[k, wi] = int((~aw).sum())
        coreinfo.append(wins)

    mA = [max(1, int(np.ceil(nA[:, w].max() / P))) for w in range(WINDOWS)]
    mB = [max(1, int(np.ceil(nB[:, w].max() / P))) for w in range(WINDOWS)]
    mW = [a + b for a, b in zip(mA, mB)]
    dims = dict(mA=mA, mB=mB, mW=mW,
                sumA=sum(mA) * P, sumB=sum(mB) * P,
                sumM=sum(mW), sumE=sum(mW) * P, mmax=max(mW))

    per_core = []
    for k in range(NCORES):
        idxA = np.zeros((16, dims["sumA"] // 16), np.int16)
        idxB = np.zeros((16, dims["sumB"] // 16), np.int16)
        dstcol = np.full((dims["sumM"], P), -1.0, bf16)
        cA = cB = cM = 0
        for w in range(WINDOWS):
            sA, dA, sB, dB = coreinfo[k][w]
            LA, LB = mA[w] * P, mB[w] * P
            idxA[:, cA // 16:(cA + LA) // 16] = _wrap_idx(sA, LA)
            idxB[:, cB // 16:(cB + LB) // 16] = _wrap_idx(sB, LB)
            dv = np.full(LA + LB, -1.0, np.float32)
            dv[: len(dA)] = dA
            dv[LA: LA + len(dB)] = dB
            dstcol[cM:cM + mW[w]] = dv.reshape(mW[w], P).astype(bf16)
            cA += LA
            cB += LB
            cM += mW[w]

        bv = np.full((OWNPAD,), -1.0, np.float32)
        bv[:OWN] = batch[k * OWN:(k + 1) * OWN].astype(np.float32)
        bvT = np.ascontiguousarray(
            bv.reshape(WINDOWS, P).T.astype(bf16))          # [P, WINDOWS]
        # own-node table row ids (for al_dst of own windows), A/B split + select
        own = np.arange(OWNPAD, dtype=np.int32) + k * OWN
        own = np.minimum(own, NODES_PAD - 1)
        selA = own < TAB_HALF
        ownA = np.where(selA, own, 0)
        ownB = np.where(selA, 0, own - TAB_HALF)
        sel = selA.astype(np.float32).reshape(WINDOWS, P).astype(bf16)
        per_core.append(dict(
            idxA=idxA, idxB=idxB, dstcol=dstcol,
            dstcolT=np.ascontiguousarray(dstcol.T),         # [P, sumM]
            batchvT=bvT,
            ownA=_wrap_idx(ownA, OWNPAD), ownB=_wrap_idx(ownB, OWNPAD),
            ownsel=sel))
    return dims, per_core


def _prep_weights(x, W1, a_src1, a_dst1, W2, a_src2, a_dst2):
    xT = np.zeros((F_IN_PAD, NODES_PAD), bf16)
    xT[:F_IN, :N_NODES] = x.T.astype(bf16)

    Asrc = np.zeros((H1, HEADS), np.float32)
    Adst = np.zeros((H1, HEADS), np.float32)
    for h in range(HEADS):
        Asrc[h * HID:(h + 1) * HID, h] = a_src1[h]
        Adst[h * HID:(h + 1) * HID, h] = a_dst1[h]
    Waug = np.zeros((F_IN_PAD, 8 + H1), np.float32)
    Waug[:F_IN, 0:4] = W1 @ Asrc
    Waug[:F_IN, 4:8] = W1 @ Adst
    Waug[:F_IN, 8:] = W1
    Waug = Waug.astype(bf16)

    W2aug = np.zeros((H1, HID + 2), np.float32)
    W2aug[:, :HID] = W2
    W2aug[:, HID] = W2 @ a_src2[0]
    W2aug[:, HID + 1] = W2 @ a_dst2[0]
    W2aug = W2aug.astype(bf16)
    return xT, Waug, W2aug


# ======================================================================
# phase A builder
# ======================================================================

def build_phase_a(dims):
    mA, mB, mW = dims["mA"], dims["mB"], dims["mW"]
    mmax = dims["mmax"]
    nc = bacc.Bacc("TRN2", target_bir_lowering=False, debug=False,
                   num_swdge_queues=4)

    xT_d = nc.dram_tensor("xT", [F_IN_PAD, NODES_PAD], BF16, kind="ExternalInput")
    Waug_d = nc.dram_tensor("Waug", [F_IN_PAD, 264], BF16, kind="ExternalInput")
    W2aug_d = nc.dram_tensor("W2aug", [H1, 66], BF16, kind="ExternalInput")
    idxA_d = nc.dram_tensor("idxA", [16, dims["sumA"] // 16], I16, kind="ExternalInput")
    idxB_d = nc.dram_tensor("idxB", [16, dims["sumB"] // 16], I16, kind="ExternalInput")
    dstcol_d = nc.dram_tensor("dstcol", [dims["sumM"], P], BF16, kind="ExternalInput")
    dstcolT_d = nc.dram_tensor("dstcolT", [P, dims["sumM"]], BF16, kind="ExternalInput")
    ownA_d = nc.dram_tensor("ownA", [16, OWNPAD // 16], I16, kind="ExternalInput")
    ownB_d = nc.dram_tensor("ownB", [16, OWNPAD // 16], I16, kind="ExternalInput")
    ownsel_d = nc.dram_tensor("ownsel", [WINDOWS, P], BF16, kind="ExternalInput")
    iotaF_d = nc.dram_tensor("iotaF", [1, P], BF16, kind="ExternalInput")
    iotaC_d = nc.dram_tensor("iotaC", [P, 1], BF16, kind="ExternalInput")
    b1_d = nc.dram_tensor("b1r", [1, H1], F32, kind="ExternalInput")

    T2own_d = nc.dram_tensor("T2own", [OWNPAD, 65], BF16, kind="ExternalOutput")
    ad2_d = nc.dram_tensor("ad2", [P, dims["sumM"]], F32, kind="ExternalOutput")

    with tile.TileContext(nc) as tc:
        ctx = contextlib.ExitStack()
        with ctx:
            dram = ctx.enter_context(tc.tile_pool(name="dram", bufs=1, space="DRAM"))
            T1a = dram.tile([TAB_A, T1_COLS], BF16)
            T1b = dram.tile([TAB_B, T1_COLS], BF16)

            const = ctx.enter_context(tc.tile_pool(name="const", bufs=1))
            waug_t = const.tile([P, 4, 264], BF16)
            nc.sync.dma_start(waug_t[:], Waug_d[:].rearrange("(ko p) c -> p ko c", p=P))
            w2aug_t = const.tile([P, 2, 66], BF16)
            nc.sync.dma_start(w2aug_t[:], W2aug_d[:].rearrange("(ko p) c -> p ko c", p=P))
            iotaF_t = const.tile([P, P], BF16)
            nc.sync.dma_start(iotaF_t[:], iotaF_d[:].to_broadcast([P, P]))
            iotaC_t = const.tile([P, 1], BF16)
            nc.sync.dma_start(iotaC_t[:], iotaC_d[:])
            b1_t = const.tile([P, H1], F32)
            nc.sync.dma_start(b1_t[:], b1_d[:].to_broadcast([P, H1]))
            ident_t = const.tile([P, P], F32)
            make_identity(nc, ident_t[:])
            ones_t = const.tile([1, P], BF16)
            nc.vector.memset(ones_t[:], 1.0)
            # own-node [as|ad] cache, filled after dense phase
            ocp = const.tile([P, WINDOWS, 8], BF16)

            # batched per-window metadata + output accumulators
            ia_all = const.tile([P, dims["sumA"] // 16], I16)
            nc.sync.dma_start(
                ia_all[:],
                idxA_d[None, :, :].to_broadcast([8, 16, dims["sumA"] // 16]))
            ib_all = const.tile([P, dims["sumB"] // 16], I16)
            nc.sync.dma_start(
                ib_all[:],
                idxB_d[None, :, :].to_broadcast([8, 16, dims["sumB"] // 16]))
            dcol_all = const.tile([P, dims["sumM"]], BF16)
            nc.sync.dma_start(dcol_all[:], dstcolT_d[:])
            t2_acc = const.tile([P, WINDOWS, 65], BF16)
            ad2_acc = const.tile([P, dims["sumM"]], F32)

            # ---------------- dense phase ----------------
            CH = 8  # node blocks per xT chunk
            with tc.tile_pool(name="dense", bufs=3) as dpool, \
                 tc.tile_pool(name="dpsum", bufs=4, space="PSUM") as dps:
                for c0 in range(0, NBLK, CH):
                    nb = min(CH, NBLK - c0)
                    nchunk = nb * P
                    xt_t = dpool.tile([P, 4, CH * P], BF16, tag="xt")
                    nc.sync.dma_start(
                        xt_t[:, :, :nchunk],
                        xT_d[:].rearrange("(ko p) n -> p ko n", p=P)[
                            :, :, c0 * P: c0 * P + nchunk],
                    )
                    tc_t = dpool.tile([P, CH, 264], BF16, tag="t1c")
                    for b in range(nb):
                        ps = dps.tile([P, 264], F32, tag="dps")
                        for ko in range(4):
                            nc.tensor.matmul(
                                ps[:],
                                lhsT=xt_t[:, ko, b * P:(b + 1) * P],
                                rhs=waug_t[:, ko, :],
                                start=(ko == 0),
                                stop=(ko == 3),
                            )
                        nc.scalar.copy(tc_t[:, b, :], ps[:])
                    # batched store, split at the A/B table boundary
                    for s, e, isA in ((c0, min(c0 + nb, TAB_ABLK), True),
                                      (max(c0, TAB_ABLK), c0 + nb, False)):
                        if s >= e:
                            continue
                        if isA:
                            dst = T1a[s * P:e * P, 0:264]
                        else:
                            dst = T1b[(s - TAB_ABLK) * P:(e - TAB_ABLK) * P, 0:264]
                        nc.sync.dma_start(
                            dst.rearrange("(b p) c -> p b c", p=P),
                            tc_t[:, s - c0:e - c0, :])

            # own [as|ad] rows via A/B gather + select (program is
            # core-independent; indices/select are per-core data)
            if SKIP_OWN:
                nc.vector.memset(ocp[:], 0.0)
            else:
              with tc.tile_pool(name="own", bufs=1) as opool:
                  oiA = opool.tile([P, OWNPAD // 16], I16, tag="oiA")
                  nc.sync.dma_start(
                      oiA[:], ownA_d[None, :, :].to_broadcast([8, 16, OWNPAD // 16]))
                  oiB = opool.tile([P, OWNPAD // 16], I16, tag="oiB")
                  nc.sync.dma_start(
                      oiB[:], ownB_d[None, :, :].to_broadcast([8, 16, OWNPAD // 16]))
                  ogA = opool.tile([P, WINDOWS, T1_COLS], BF16, tag="ogA")
                  nc.gpsimd.dma_gather(
                      out_ap=ogA[:], in_ap=T1a[:], idxs_ap=oiA[:],
                      num_idxs=OWNPAD, num_idxs_reg=OWNPAD, elem_size=T1_COLS,
                      single_packet=False, queue_num=1)
                  ogB = opool.tile([P, WINDOWS, T1_COLS], BF16, tag="ogB")
                  nc.gpsimd.dma_gather(
                      out_ap=ogB[:], in_ap=T1b[:], idxs_ap=oiB[:],
                      num_idxs=OWNPAD, num_idxs_reg=OWNPAD, elem_size=T1_COLS,
                      single_packet=False, queue_num=2)
                  osel = opool.tile([P, WINDOWS], BF16, tag="osel")
                  nc.sync.dma_start(osel[:], ownsel_d[:].rearrange("j p -> p j"))
                  oinv = opool.tile([P, WINDOWS], BF16, tag="oinv")
                  nc.vector.tensor_scalar(
                      oinv[:], osel[:], -1.0, 1.0, ALU.mult, ALU.add)
                  tmpA = opool.tile([P, WINDOWS, 8], BF16, tag="tmpA")
                  nc.vector.tensor_tensor(
                      tmpA[:], ogA[:, :, 0:8],
                      osel[:, :, None].to_broadcast([P, WINDOWS, 8]), ALU.mult)
                  tmpB = opool.tile([P, WINDOWS, 8], BF16, tag="tmpB")
                  nc.vector.tensor_tensor(
                      tmpB[:], ogB[:, :, 0:8],
                      oinv[:, :, None].to_broadcast([P, WINDOWS, 8]), ALU.mult)
                  nc.vector.tensor_tensor(ocp[:], tmpA[:], tmpB[:], ALU.add)

            # ---------------- window loop (layer 1 + dense 2) ----------------
            wpool = ctx.enter_context(tc.tile_pool(name="win", bufs=3))
            spool = ctx.enter_context(tc.tile_pool(name="small", bufs=2))
            ps_dr = ctx.enter_context(tc.tile_pool(name="psdr", bufs=2, space="PSUM"))
            ps_ad1 = ctx.enter_context(tc.tile_pool(name="psad1", bufs=1, space="PSUM"))
            ps_agg = ctx.enter_context(tc.tile_pool(name="psagg", bufs=2, space="PSUM"))
            ps_z1t = ctx.enter_context(tc.tile_pool(name="psz1t", bufs=1, space="PSUM"))
            ps_h2 = ctx.enter_context(tc.tile_pool(name="psh2", bufs=1, space="PSUM"))
            ps_ad2 = ctx.enter_context(tc.tile_pool(name="psad2", bufs=1, space="PSUM"))

            cA = cB = cM = cE = 0
            for w in range(WINDOWS):
                ma, mb, m = mA[w], mB[w], mW[w]
                Ew = m * P
                if w >= MAXWIN:
                    cA += ma * P; cB += mb * P; cM += m; cE += Ew
                    continue

                # --- gathers (idx slices come from the batched tables) ---
                v_t = wpool.tile([P, mmax, T1_COLS], BF16, tag="v")
                nc.gpsimd.dma_gather(
                    out_ap=v_t[:, 0:ma, :], in_ap=T1a[:],
                    idxs_ap=ia_all[:, cA // 16:(cA + ma * P) // 16],
                    num_idxs=ma * P, num_idxs_reg=ma * P, elem_size=T1_COLS,
                    single_packet=False, queue_num=w % 4)
                nc.gpsimd.dma_gather(
                    out_ap=v_t[:, ma:m, :], in_ap=T1b[:],
                    idxs_ap=ib_all[:, cB // 16:(cB + mb * P) // 16],
                    num_idxs=mb * P, num_idxs_reg=mb * P, elem_size=T1_COLS,
                    single_packet=False, queue_num=(w + 2) % 4)
                drow_t = wpool.tile([1, mmax * P], BF16, tag="drow")
                nc.sync.dma_start(
                    drow_t[:, :Ew],
                    dstcol_d[cM:cM + m, :].rearrange("j p -> (j p)")[None, :])

                # --- S (edge-major indicator) ---
                s_t = wpool.tile([P, mmax, P], BF16, tag="s")
                nc.vector.tensor_tensor(
                    s_t[:, :m, :],
                    dcol_all[:, cM:cM + m, None].to_broadcast([P, m, P]),
                    iotaF_t[:, None, :].to_broadcast([P, m, P]),
                    ALU.is_equal)
                # --- S_T (dst-major indicator) via PE row-broadcast ---
                drb_t = wpool.tile([P, mmax * P], BF16, tag="drb")
                for c0 in range(0, Ew, 512):
                    cw = min(512, Ew - c0)
                    psd = ps_dr.tile([P, 512], F32, tag="psdr")
                    nc.tensor.matmul(
                        psd[:, :cw], lhsT=ones_t[:], rhs=drow_t[:, c0:c0 + cw],
                        start=True, stop=True)
                    nc.scalar.copy(drb_t[:, c0:c0 + cw], psd[:, :cw])
                str_t = wpool.tile([P, mmax * P], BF16, tag="str")
                nc.vector.tensor_tensor(
                    str_t[:, :Ew],
                    iotaC_t[:].to_broadcast([P, Ew]),
                    drb_t[:, :Ew],
                    ALU.is_equal)

                # --- ad1 per edge ---
                pad1 = ps_ad1.tile([P, 4 * mmax], F32, tag="psad1")
                for j in range(m):
                    nc.tensor.matmul(
                        pad1[:, j * 4:(j + 1) * 4],
                        lhsT=str_t[:, j * P:(j + 1) * P],
                        rhs=ocp[:, w, 4:8],
                        start=True, stop=True)
                # --- ex = exp(lrelu(as + ad)) (padding edges are zeroed by
                # the all-zero indicator column in s_t, no mask needed) ---
                zf = spool.tile([P, mmax, 4], F32, tag="zf")
                nc.vector.tensor_tensor(
                    zf[:, :m, :], v_t[:, :m, 0:4],
                    pad1[:].rearrange("p (j c) -> p j c", c=4)[:, :m, :],
                    ALU.add)
                zt = spool.tile([P, mmax, 4], F32, tag="zt")
                nc.vector.tensor_scalar_mul(zt[:, :m, :], zf[:, :m, :], NEG_SLOPE)
                nc.vector.tensor_tensor(zt[:, :m, :], zt[:, :m, :], zf[:, :m, :],
                                        ALU.max)
                ex_t = spool.tile([P, mmax, 4], BF16, tag="ex")
                nc.scalar.activation(ex_t[:, :m, :], zt[:, :m, :], AF.Exp)
                # --- Vw = [h*ex | ex] ---
                vw_t = wpool.tile([P, mmax, 260], BF16, tag="vw")
                nc.vector.tensor_tensor(
                    vw_t[:, :m, 0:256].rearrange("p m (h c) -> p m h c", h=HEADS),
                    v_t[:, :m, 8:264].rearrange("p m (h c) -> p m h c", h=HEADS),
                    ex_t[:, :m, :, None].to_broadcast([P, m, HEADS, HID]),
                    ALU.mult)
                nc.vector.tensor_copy(vw_t[:, :m, 256:260], ex_t[:, :m, :])

                # --- aggregate ---
                pagg = ps_agg.tile([P, 260], F32, tag="psagg")
                for j in range(m):
                    nc.tensor.matmul(
                        pagg[:], lhsT=s_t[:, j, :], rhs=vw_t[:, j, :],
                        start=(j == 0), stop=(j == m - 1))
                # --- out1 = agg / s + b1 ; z1 = relu ---
                sden = spool.tile([P, 4], F32, tag="sden")
                nc.vector.tensor_scalar_add(sden[:], pagg[:, 256:260], EPS)
                nc.vector.reciprocal(sden[:], sden[:])
                z1 = spool.tile([P, H1], F32, tag="z1")
                nc.vector.tensor_tensor(
                    z1[:].rearrange("p (h c) -> p h c", h=HEADS),
                    pagg[:, 0:256].rearrange("p (h c) -> p h c", h=HEADS),
                    sden[:, :, None].to_broadcast([P, HEADS, HID]),
                    ALU.mult)
                nc.vector.tensor_add(z1[:], z1[:], b1_t[:])
                nc.scalar.activation(z1[:], z1[:], AF.Relu)

                # --- dense 2: h2aug = z1 @ W2aug ---
                z1t = spool.tile([P, 2, P], BF16, tag="z1t")
                for hh in range(2):
                    pzt = ps_z1t.tile([P, P], F32, tag="psz1t")
                    nc.tensor.transpose(
                        pzt[:], z1[:, hh * P:(hh + 1) * P], ident_t[:])
                    nc.scalar.copy(z1t[:, hh, :], pzt[:])
                ph2 = ps_h2.tile([P, 66], F32, tag="psh2")
                for hh in range(2):
                    nc.tensor.matmul(
                        ph2[:], lhsT=z1t[:, hh, :], rhs=w2aug_t[:, hh, :],
                        start=(hh == 0), stop=(hh == 1))
                nc.scalar.copy(t2_acc[:, w, 0:1], ph2[:, 64:65])
                nc.scalar.copy(t2_acc[:, w, 1:65], ph2[:, 0:64])

                # --- ad2 per edge (for phase B) ---
                ald2 = spool.tile([P, 1], BF16, tag="ald2")
                nc.scalar.copy(ald2[:], ph2[:, 65:66])
                pad2 = ps_ad2.tile([P, mmax], F32, tag="psad2")
                for j in range(m):
                    nc.tensor.matmul(
                        pad2[:, j:j + 1],
                        lhsT=str_t[:, j * P:(j + 1) * P],
                        rhs=ald2[:], start=True, stop=True)
                nc.vector.tensor_copy(ad2_acc[:, cM:cM + m], pad2[:, :m])

                cA += ma * P
                cB += mb * P
                cM += m
                cE += Ew

            # single batched stores of the accumulated outputs
            nc.sync.dma_start(
                T2own_d[:].rearrange("(w p) c -> p w c", p=P), t2_acc[:])
            nc.sync.dma_start(ad2_d[:], ad2_acc[:])

    nc.compile()
    return nc


# ======================================================================
# phase B builder
# ======================================================================

def build_phase_b(dims):
    mA, mB, mW = dims["mA"], dims["mB"], dims["mW"]
    mmax = dims["mmax"]
    nc = bacc.Bacc("TRN2", target_bir_lowering=False, debug=False,
                   num_swdge_queues=4)

    T2A_d = nc.dram_tensor("T2A", [TAB_A, T2_COLS], BF16, kind="ExternalInput")
    T2B_d = nc.dram_tensor("T2B", [TAB_B, T2_COLS], BF16, kind="ExternalInput")
    idxA_d = nc.dram_tensor("idxA", [16, dims["sumA"] // 16], I16, kind="ExternalInput")
    idxB_d = nc.dram_tensor("idxB", [16, dims["sumB"] // 16], I16, kind="ExternalInput")
    dstcolT_d = nc.dram_tensor("dstcolT", [P, dims["sumM"]], BF16, kind="ExternalInput")
    ad2_d = nc.dram_tensor("ad2", [P, dims["sumM"]], F32, kind="ExternalInput")
    iotaF_d = nc.dram_tensor("iotaF", [1, P], BF16, kind="ExternalInput")
    giota_d = nc.dram_tensor("giota", [1, N_GRAPHS], BF16, kind="ExternalInput")
    batchvT_d = nc.dram_tensor("batchvT", [P, WINDOWS], BF16, kind="ExternalInput")
    b2_d = nc.dram_tensor("b2r", [1, HID], F32, kind="ExternalInput")
    cnt_d = nc.dram_tensor("cnt", [N_GRAPHS, 1], F32, kind="ExternalInput")
    Wl_d = nc.dram_tensor("Wl", [HID, 2], F32, kind="ExternalInput")

    out_d = nc.dram_tensor("partial", [N_GRAPHS, 2], F32, kind="ExternalOutput")

    with tile.TileContext(nc) as tc:
        ctx = contextlib.ExitStack()
        with ctx:
            const = ctx.enter_context(tc.tile_pool(name="const", bufs=1))
            iotaF_t = const.tile([P, P], BF16)
            nc.sync.dma_start(iotaF_t[:], iotaF_d[:].to_broadcast([P, P]))
            giota_t = const.tile([P, N_GRAPHS], BF16)
            nc.sync.dma_start(giota_t[:], giota_d[:].to_broadcast([P, N_GRAPHS]))
            b2_t = const.tile([P, HID], F32)
            nc.sync.dma_start(b2_t[:], b2_d[:].to_broadcast([P, HID]))
            cnt_t = const.tile([N_GRAPHS, 1], F32)
            nc.sync.dma_start(cnt_t[:], cnt_d[:])
            wl_t = const.tile([P, 2], F32)
            nc.vector.memset(wl_t[:], 0.0)
            nc.sync.dma_start(wl_t[:HID, :], Wl_d[:])
            ident_t = const.tile([P, P], F32)
            make_identity(nc, ident_t[:])
            pts = const.tile([P, N_GRAPHS], F32)
            nc.vector.memset(pts[:], 0.0)

            ia_all = const.tile([P, dims["sumA"] // 16], I16)
            nc.sync.dma_start(
                ia_all[:],
                idxA_d[None, :, :].to_broadcast([8, 16, dims["sumA"] // 16]))
            ib_all = const.tile([P, dims["sumB"] // 16], I16)
            nc.sync.dma_start(
                ib_all[:],
                idxB_d[None, :, :].to_broadcast([8, 16, dims["sumB"] // 16]))
            dcol_all = const.tile([P, dims["sumM"]], BF16)
            nc.sync.dma_start(dcol_all[:], dstcolT_d[:])
            ad2_all = const.tile([P, dims["sumM"]], F32)
            nc.sync.dma_start(ad2_all[:], ad2_d[:])
            bv_all = const.tile([P, WINDOWS], BF16)
            nc.sync.dma_start(bv_all[:], batchvT_d[:])

            wpool = ctx.enter_context(tc.tile_pool(name="win", bufs=3))
            spool = ctx.enter_context(tc.tile_pool(name="small", bufs=2))
            ps_agg = ctx.enter_context(tc.tile_pool(name="psagg", bufs=2, space="PSUM"))
            ps_pool = ctx.enter_context(tc.tile_pool(name="pspool", bufs=1, space="PSUM"))
            ps_fin = ctx.enter_context(tc.tile_pool(name="psfin", bufs=1, space="PSUM"))

            ppool = ps_pool.tile([N_GRAPHS, HID], F32)

            cA = cB = cM = cE = 0
            for w in range(WINDOWS):
                ma, mb, m = mA[w], mB[w], mW[w]
                Ew = m * P

                v_t = wpool.tile([P, mmax, T2_COLS], BF16, tag="v")
                nc.gpsimd.dma_gather(
                    out_ap=v_t[:, 0:ma, :], in_ap=T2A_d[:],
                    idxs_ap=ia_all[:, cA // 16:(cA + ma * P) // 16],
                    num_idxs=ma * P, num_idxs_reg=ma * P, elem_size=T2_COLS,
                    single_packet=False, queue_num=w % 4)
                nc.gpsimd.dma_gather(
                    out_ap=v_t[:, ma:m, :], in_ap=T2B_d[:],
                    idxs_ap=ib_all[:, cB // 16:(cB + mb * P) // 16],
                    num_idxs=mb * P, num_idxs_reg=mb * P, elem_size=T2_COLS,
                    single_packet=False, queue_num=(w + 2) % 4)

                s_t = wpool.tile([P, mmax, P], BF16, tag="s")
                nc.vector.tensor_tensor(
                    s_t[:, :m, :],
                    dcol_all[:, cM:cM + m, None].to_broadcast([P, m, P]),
                    iotaF_t[:, None, :].to_broadcast([P, m, P]),
                    ALU.is_equal)
                as2f = spool.tile([P, mmax], F32, tag="as2f")
                nc.scalar.copy(as2f[:, :m], v_t[:, :m, 0])
                zf = spool.tile([P, mmax], F32, tag="zf")
                nc.vector.tensor_tensor(
                    zf[:, :m], as2f[:, :m], ad2_all[:, cM:cM + m], ALU.add)
                zt = spool.tile([P, mmax], F32, tag="zt")
                nc.vector.tensor_scalar_mul(zt[:, :m], zf[:, :m], NEG_SLOPE)
                nc.vector.tensor_tensor(zt[:, :m], zt[:, :m], zf[:, :m], ALU.max)
                ex_t = spool.tile([P, mmax], BF16, tag="ex")
                nc.scalar.activation(ex_t[:, :m], zt[:, :m], AF.Exp)

                vw_t = wpool.tile([P, mmax, 65], BF16, tag="vw")
                nc.vector.tensor_tensor(
                    vw_t[:, :m, 0:64],
                    v_t[:, :m, 1:65],
                    ex_t[:, :m, None].to_broadcast([P, m, HID]),
                    ALU.mult)
                nc.vector.tensor_copy(vw_t[:, :m, 64:65], ex_t[:, :m, None])

                pagg = ps_agg.tile([P, 65], F32, tag="psagg")
                for j in range(m):
                    nc.tensor.matmul(
                        pagg[:], lhsT=s_t[:, j, :], rhs=vw_t[:, j, :],
                        start=(j == 0), stop=(j == m - 1))
                sden = spool.tile([P, 1], F32, tag="sden")
                nc.vector.tensor_scalar_add(sden[:], pagg[:, 64:65], EPS)
                nc.vector.reciprocal(sden[:], sden[:])
                z2 = spool.tile([P, HID], F32, tag="z2")
                nc.vector.tensor_tensor(
                    z2[:], pagg[:, 0:64], sden[:].to_broadcast([P, HID]),
                    ALU.mult)
                nc.vector.tensor_add(z2[:], z2[:], b2_t[:])
                z2b = spool.tile([P, HID], BF16, tag="z2b")
                nc.scalar.activation(z2b[:], z2[:], AF.Relu)

                pw_t = spool.tile([P, N_GRAPHS], BF16, tag="pw")
                nc.vector.tensor_tensor(
                    pw_t[:], bv_all[:, w:w + 1].to_broadcast([P, N_GRAPHS]),
                    giota_t[:], ALU.is_equal)
                nc.tensor.matmul(
                    ppool[:], lhsT=pw_t[:], rhs=z2b[:],
                    start=(w == 0), stop=(w == WINDOWS - 1))
                cA += ma * P
                cB += mb * P
                cM += m
                cE += Ew

            # pooled partial logits
            crec = spool.tile([N_GRAPHS, 1], F32, tag="crec")
            nc.vector.reciprocal(crec[:], cnt_t[:])
            pooled = spool.tile([N_GRAPHS, HID], F32, tag="pooled")
            nc.vector.tensor_tensor(
                pooled[:], ppool[:], crec[:].to_broadcast([N_GRAPHS, HID]),
                ALU.mult)
            ptp = ps_fin.tile([HID, N_GRAPHS], F32)
            nc.tensor.transpose(ptp[:], pooled[:], ident_t[:N_GRAPHS, :N_GRAPHS])
            nc.vector.tensor_copy(pts[:HID, :], ptp[:])
            plog = ps_fin.tile([N_GRAPHS, 2], F32)
            nc.tensor.matmul(plog[:], lhsT=pts[:], rhs=wl_t[:],
                             start=True, stop=True)
            outs = spool.tile([N_GRAPHS, 2], F32, tag="outs")
            nc.vector.tensor_copy(outs[:], plog[:])
            nc.sync.dma_start(out_d[:], outs[:])

    nc.compile()
    return nc


# ======================================================================
# driver
# ======================================================================

def _run(nc, in_maps, label):
    res = bass_utils.run_bass_kernel_spmd(
        nc, in_maps, core_ids=list(range(NCORES)), trace=TRACE)
    if TRACE:
        LAST_TIMES[label] = res.exec_time_ns
    return res.results


def kernel(x, edge_index, batch, W1, a_src1, a_dst1, b1,
           W2, a_src2, a_dst2, b2, Wl, bl):
    if TRACE:
        try:
            import axon_shim  # noqa: F401
        except ImportError:
            pass

    x = np.asarray(x, np.float32)
    edge_index = np.asarray(edge_index)
    batch = np.asarray(batch)

    key = hashlib.sha1(edge_index.tobytes() + batch.tobytes()).hexdigest()
    if key in _CACHE:
        dims, per_core, nc_a, nc_b = _CACHE[key]
    else:
        dims, per_core = _prep(edge_index, batch)
        nc_a = build_phase_a(dims)
        nc_b = build_phase_b(dims)
        _CACHE[key] = (dims, per_core, nc_a, nc_b)

    xT, Waug, W2aug = _prep_weights(
        x, np.asarray(W1, np.float32), np.asarray(a_src1, np.float32),
        np.asarray(a_dst1, np.float32), np.asarray(W2, np.float32),
        np.asarray(a_src2, np.float32), np.asarray(a_dst2, np.float32))

    iotaF = np.arange(P, dtype=np.float32).astype(bf16)[None, :]
    iotaC = np.arange(P, dtype=np.float32).astype(bf16)[:, None]
    giota = np.arange(N_GRAPHS, dtype=np.float32).astype(bf16)[None, :]
    b1r = np.asarray(b1, np.float32)[None, :]
    b2r = np.asarray(b2, np.float32)[None, :]
    cnt = np.maximum(
        np.bincount(np.asarray(batch).astype(np.int64), minlength=N_GRAPHS), 1
    ).astype(np.float32)[:, None]
    Wl32 = np.asarray(Wl, np.float32)
    bl32 = np.asarray(bl, np.float32)

    in_maps_a = []
    for k in range(NCORES):
        pc = per_core[k]
        in_maps_a.append(dict(
            xT=xT, Waug=Waug, W2aug=W2aug,
            idxA=pc["idxA"], idxB=pc["idxB"],
            dstcol=pc["dstcol"], dstcolT=pc["dstcolT"],
            ownA=pc["ownA"], ownB=pc["ownB"], ownsel=pc["ownsel"],
            iotaF=iotaF, iotaC=iotaC, b1r=b1r,
        ))
    res_a = _run(nc_a, in_maps_a, "phase_a")

    T2 = np.zeros((NODES_PAD, T2_COLS), bf16)
    for k in range(NCORES):
        T2[k * OWN:(k + 1) * OWN, 0:65] = res_a[k]["T2own"][:OWN, :]
    T2A, T2B = T2[:TAB_A], T2[TAB_A:]

    in_maps_b = []
    for k in range(NCORES):
        pc = per_core[k]
        in_maps_b.append(dict(
            T2A=T2A, T2B=T2B,
            idxA=pc["idxA"], idxB=pc["idxB"],
            dstcolT=pc["dstcolT"],
            ad2=res_a[k]["ad2"],
            iotaF=iotaF, giota=giota,
            batchvT=pc["batchvT"], b2r=b2r, cnt=cnt, Wl=Wl32,
        ))
    res_b = _run(nc_b, in_maps_b, "phase_b")

    out = np.zeros((N_GRAPHS, 2), np.float32)
    for k in range(NCORES):
        out += res_b[k]["partial"]
    out += bl32[None, :]
    return out
